# revision 42
# baseline (speedup 1.0000x reference)
"""Multi-head attention (with the repo's k=q bug) on 8 Trainium2 NeuronCores.

Reference computation (B=2, S=2048, D=512, H=8, DK=64):
    q = query @ Wq.T ; v = value @ Wv.T          (k-projection is dead code)
    qh = q.reshape(B, H, S, DK)  (raw view: head h = a contiguous 256-row slab
                                  of q, re-chunked into rows of 64)
    kh = qh                      (repo bug: key = query.view(...))
    scores = qh @ qh^T / 8 ; mask ; softmax ; x = attn @ vh
    out = x.transpose/reshape @ Wo.T

Sharding: core c owns head h=c for both batches (2 (b,h) pairs/core).
Per pair the device computes, in score-TRANSPOSED layout S_T[j, i]:
    E_T = exp(S_T/8 - 20) * mask_T        (mask synthesized host-side; only
                                           diagonal blocks need one)
    [x_unnorm^T; l] = [vh | ones]^T-style augmented PV matmul
    po = x_unnorm^T.T @ Wo_h.T            (unnormalized partial output)
The host divides po rows by l and sums partials over heads/cores.
The causal structure of the mask (verified exactly on the host) lets the
device skip all fully-masked j-tiles. Non-causal masks fall back to numpy.
"""

import math
import sys

import numpy as np

sys.path.insert(0, "/opt/trn_rl_repo")

B, S, D, H, DK = 2, 2048, 512, 8, 64
NCORES = 8
SLAB = S // H          # 256 query rows per head-slab
CHUNK = 512            # i-chunk width (PSUM bank)
JT = 128               # j-tile height
NCHUNK = S // CHUNK    # 4
NJT = S // JT          # 16
EXP_BIAS = -20.0
RESTRICT_S = False
RESTRICT_PV = False       # exp(s/8 - 20): overflow headroom; cancels in l-division


def _enable_ldw_opt():
    """Flip walrus --enable-ldw-opt: elides back-to-back LDWEIGHTS of the
    same stationary operand, which the t-grouped matmul order produces."""
    from concourse import bass_utils
    if getattr(bass_utils, "_ldw_patched", False):
        return
    orig = bass_utils.run_command

    def run_command(argv, **kw):
        argv = [a.replace("--enable-ldw-opt=false", "--enable-ldw-opt=true")
                if isinstance(a, str) else a for a in argv]
        return orig(argv, **kw)

    bass_utils.run_command = run_command
    bass_utils._ldw_patched = True

_cache: dict = {}


def _build_causal():
    import concourse.bass as bass
    import concourse.tile as tile
    from concourse import bacc, mybir

    f32 = mybir.dt.float32
    bf16 = mybir.dt.bfloat16
    nc = bacc.Bacc("TRN2", target_bir_lowering=False, debug=False,
                   num_devices=NCORES)

    # inputs are host-prearranged partition-major ([128, k, o] contiguous)
    # so each input loads with one DMA of 2-4KB-per-partition descriptors
    KTc = D // 128
    qT = nc.dram_tensor("qT", [B, 128, KTc, SLAB], bf16,
                        kind="ExternalInput").ap()
    vT = nc.dram_tensor("vT", [B, 128, KTc, SLAB], bf16,
                        kind="ExternalInput").ap()
    wqT = nc.dram_tensor("wqT", [128, KTc, D], bf16,
                         kind="ExternalInput").ap()
    wvT = nc.dram_tensor("wvT", [128, KTc, D], bf16,
                         kind="ExternalInput").ap()
    woT = nc.dram_tensor("woT", [DK, D], bf16, kind="ExternalInput").ap()
    mpat = nc.dram_tensor("mpat", [JT, 4, CHUNK], mybir.dt.bfloat16,
                          kind="ExternalInput").ap()
    po = nc.dram_tensor("po", [B, S, D], bf16, kind="ExternalOutput").ap()
    lo = nc.dram_tensor("lo", [B, 1, S], f32, kind="ExternalOutput").ap()

    KT = D // 128  # 4 k-tiles over the contraction dim of the projections

    def mm(out, lhsT, rhs, **kw):
        nc.tensor.matmul(out, lhsT, rhs, **kw)

    def r(ap):
        return ap

    with tile.TileContext(nc) as tc:
        with (
            tc.tile_pool(name="const", bufs=1) as constp,
            tc.tile_pool(name="acts", bufs=2) as actp,
            tc.tile_pool(name="qhT", bufs=2) as qhTp,
            tc.tile_pool(name="vh", bufs=2) as vhp,
            tc.tile_pool(name="eT", bufs=4) as eTp,
            tc.tile_pool(name="xT", bufs=2) as xTp,
            tc.tile_pool(name="fo", bufs=3) as fop,
            tc.tile_pool(name="psS", bufs=2, space="PSUM") as psS,
            tc.tile_pool(name="psX", bufs=2, space="PSUM") as psX,
            tc.tile_pool(name="psM", bufs=2, space="PSUM") as psM,
        ):
            # --- all DMA loads upfront, in first-needed order ------------
            # sync: wq k-tiles; scalar: qT (bi=0) -- the first projection's
            # inputs land first; everything else queues behind.
            wq_sb, wv_sb = [], []
            qT_sbs = [[None] * KT for _ in range(B)]
            vT_sbs = [[None] * KT for _ in range(B)]
            # batched input loads: one large DMA per tensor, host layouts
            # are already partition-major so descriptors are 2-4KB runs
            wqa = constp.tile([128, KT, D], bf16, tag="wqa")
            nc.sync.dma_start(wqa[:], wqT[:])
            wq_sb = [wqa[:, k, :] for k in range(KT)]
            qta0 = actp.tile([128, KT, SLAB], bf16, tag="qta")
            nc.scalar.dma_start(qta0[:], qT[0])
            qT_sbs[0] = [qta0[:, k, :] for k in range(KT)]
            wva = constp.tile([128, KT, D], bf16, tag="wva")
            nc.gpsimd.dma_start(wva[:], wvT[:])
            wv_sb = [wva[:, k, :] for k in range(KT)]
            vta0 = actp.tile([128, KT, SLAB], bf16, tag="vta")
            nc.scalar.dma_start(vta0[:], vT[0])
            vT_sbs[0] = [vta0[:, k, :] for k in range(KT)]
            qta1 = actp.tile([128, KT, SLAB], bf16, tag="qta")
            nc.sync.dma_start(qta1[:], qT[1])
            qT_sbs[1] = [qta1[:, k, :] for k in range(KT)]
            vta1 = actp.tile([128, KT, SLAB], bf16, tag="vta")
            nc.scalar.dma_start(vta1[:], vT[1])
            vT_sbs[1] = [vta1[:, k, :] for k in range(KT)]
            # Wo^T slice duplicated into both partition halves (row-packing)
            wo_sb = constp.tile([128, D], bf16, tag="wo")
            nc.gpsimd.dma_start(r(wo_sb[0:64, :]), r(woT[:, :]))
            nc.gpsimd.dma_start(r(wo_sb[64:128, :]), r(woT[:, :]))
            mpa = constp.tile([JT, 4, CHUNK], bf16, tag="mpa")
            nc.gpsimd.dma_start(mpa[:], mpat[:])
            mp_sb = [mpa[:, s_, :] for s_ in range(4)]
            exp_bias = constp.tile([128, 1], f32, tag="ebias")
            nc.gpsimd.memset(exp_bias[:], EXP_BIAS)

            # --- projection emitters ---------------------------------------
            # PERMUTED sequence order: storage position n*512 + jj*64 + r
            # holds seq = 512n + 8r + jj (chunk-local permutation). Makes
            # every matmul operand a contiguous slice and the PSUM->SBUF
            # casts near-contiguous (4 runs of 64). Host unpermutes rows.
            def q_proj_part(bi, qhT, jgs):
                qT_sb = qT_sbs[bi]
                qhT_v = qhT.rearrange("p (n j r) -> p n j r", n=NCHUNK, j=H)
                # dup for K=64 row-packing: scalar is idle during the b0 ramp
                # but exp-bound once attention runs (b1's projections)
                for jg in jgs:
                    ps = psM.tile([128, SLAB], f32, tag="psm")
                    for k in range(KT):
                        mm(
                            ps[:], wq_sb[k][:, 128 * jg:128 * (jg + 1)],
                            qT_sb[k][:], start=(k == 0), stop=(k == KT - 1))
                    for jl in range(2):
                        jj = 2 * jg + jl
                        nc.vector.tensor_copy(
                            qhT_v[0:64, :, jj, :], ps[64 * jl:64 * jl + 64, :])
                        if bi == 0:
                            # parallel PSUM read on the (ramp-idle) scalar
                            nc.scalar.copy(
                                qhT_v[64:128, :, jj, :],
                                ps[64 * jl:64 * jl + 64, :])
                        else:
                            nc.vector.tensor_copy(
                                qhT_v[64:128, :, jj, :], qhT_v[0:64, :, jj, :])
                return qhT

            def v_proj_part(bi, vhc, rh):
                # v slab half rh, then chunk-combined vh tiles for chunks
                # nj = 2*rh, 2*rh+1: [128, 4*(DK+1)], tile t block at cols
                # 65*(t%4); partition p = jl*64 + r_l holds
                # seq j = 512*(t//4) + 8*(p%64) + 2*(t%4) + p//64
                vT_sb = vT_sbs[bi]
                psv = psM.tile([128, D], f32, tag="psm")
                for k in range(KT):
                    mm(
                        psv[:], vT_sb[k][:, 128 * rh:128 * (rh + 1)],
                        wv_sb[k][:], start=(k == 0), stop=(k == KT - 1))
                vsl = actp.tile([128, D], bf16, tag=f"vsl{rh}")
                nc.vector.tensor_copy(vsl[:], psv[:])
                src = vsl.rearrange("p (s x) -> p s x", s=4)
                for nj in (2 * rh, 2 * rh + 1):
                    vc = vhp.tile([128, 4 * (DK + 1)], bf16, tag=f"vhc{nj}")
                    vc_r = vc.rearrange("p (t k) -> p t k", t=4)
                    nc.gpsimd.memset(vc_r[:, :, DK:DK + 1], 1.0)
                    row0 = 64 * (nj % 2)
                    nc.gpsimd.dma_start(
                        vc_r[0:64, :, 0:DK],
                        src[row0:row0 + 64, :, 0:64])
                    nc.gpsimd.dma_start(
                        vc_r[64:128, :, 0:DK],
                        src[row0:row0 + 64, :, 64:128])
                    vhc.append(vc)
                return vhc

            # --- pipelined attention over both batches --------------------
            # Pairs of j-tiles flow through: scores+exp+mask (sem) one pair
            # ahead of the PV matmuls, so the next chunk's exp is issued
            # before the current chunk's epilogue blocks the tensor queue.
            qhTs, vhcs = {}, {}
            eTs, psxs = {}, {}

            def sem(bi, n, w):
                qhT = qhTs[bi]
                pss = psS.tile([128, 2 * CHUNK], f32, tag="pss")
                for tw in range(2):
                    t_ = w + tw
                    p0 = 64 * tw
                    mm(
                        pss[:, CHUNK * tw:CHUNK * (tw + 1)],
                        qhT[p0:p0 + 64, JT * t_:JT * (t_ + 1)],
                        qhT[p0:p0 + 64, CHUNK * n:CHUNK * (n + 1)],
                        start=True, stop=True,
                        tile_position=(p0, 0))
                eT = eTp.tile([128, 2 * CHUNK], bf16, tag="eT")
                nc.scalar.activation(
                    eT[:], pss[:],
                    mybir.ActivationFunctionType.Exp,
                    bias=exp_bias[:], scale=1.0 / math.sqrt(DK))
                for tw in range(2):
                    s_ = w + tw - 4 * n
                    if s_ >= 0:
                        sl = eT[:, CHUNK * tw:CHUNK * (tw + 1)]
                        nc.vector.tensor_mul(sl, sl, mp_sb[s_][:])
                eTs[(bi, n, w)] = eT

            def pv(bi, n, w):
                n_t = 4 * n + 4
                if (bi, n) not in psxs:
                    psx = psX.tile([DK + 1, CHUNK], f32, tag="psx")
                    psxs[(bi, n)] = psx
                psx = psxs[(bi, n)]
                eT = eTs.pop((bi, n, w))
                for tw in range(2):
                    t_ = w + tw
                    vt = vhcs[bi][t_ // 4]
                    c0 = (DK + 1) * (t_ % 4)
                    nc.tensor.matmul(
                        psx[:],
                        vt[:, c0:c0 + DK + 1],
                        eT[:, CHUNK * tw:CHUNK * (tw + 1)],
                        start=(t_ == 0), stop=(t_ == n_t - 1),
                        skip_group_check=True)

            def epi(bi, n):
                psx = psxs.pop((bi, n))
                xT = xTp.tile([128, CHUNK], bf16, tag="xT")
                nc.vector.tensor_copy(xT[0:64, :], psx[0:64, :])
                nc.vector.tensor_copy(xT[64:128, :], xT[0:64, :])
                lsb = xTp.tile([1, CHUNK], f32, tag="lsb")
                nc.vector.tensor_copy(lsb[:], psx[64:65, :])
                nc.gpsimd.dma_start(
                    lo[bi, :, CHUNK * n:CHUNK * (n + 1)], lsb[:])
                for u in range(CHUNK // 128):
                    p0 = 64 * (u % 2)
                    psf = psM.tile([128, D], f32, tag="psm")
                    mm(
                        psf[:], xT[p0:p0 + 64, 128 * u:128 * (u + 1)],
                        wo_sb[p0:p0 + 64, :], start=True, stop=True,
                        tile_position=(p0, 0))
                    fo = fop.tile([128, D], bf16, tag="fo")
                    nc.vector.tensor_copy(fo[:], psf[:])
                    r0 = CHUNK * n + 128 * u
                    q = nc.sync if u % 2 == 0 else nc.gpsimd
                    q.dma_start(po[bi, r0:r0 + 128, :], fo[:])

            # b1's projections are emitted in small slices between b0's later
            # (bigger) attention chunks so the exp stream can hide the bursts
            qhT_b1 = qhTp.tile([128, S], bf16, tag="qhT")
            qhTs[1] = qhT_b1
            vhcs[1] = []
            inject = {
                (0, 0): lambda: v_proj_part(0, vhcs[0], 1),
                (0, 1): lambda: q_proj_part(1, qhTs[1], [0, 1]),
                (0, 2): lambda: (q_proj_part(1, qhTs[1], [2, 3]),
                                 v_proj_part(1, vhcs[1], 0)),
                (0, 3): lambda: v_proj_part(1, vhcs[1], 1),
            }
            qhT_b0 = qhTp.tile([128, S], bf16, tag="qhT")
            qhTs[0] = qhT_b0
            q_proj_part(0, qhTs[0], [0, 1, 2, 3])
            vhcs[0] = []
            # b1 starts with chunk 1 so the b0->b1 boundary (where b1's
            # leftover projections execute) has a meatier exp stream
            pairs = [(0, n, w) for n in (0, 1, 2, 3)
                     for w in range(0, 4 * n + 4, 2)]
            pairs += [(1, n, w) for n in (1, 2, 3, 0)
                      for w in range(0, 4 * n + 4, 2)]
            # first scores are emitted ahead of the v-projection so the exp
            # stream starts as soon as qhT is ready; sems run 1 pair ahead
            sem(*pairs[0])
            sem(*pairs[1])
            v_proj_part(0, vhcs[0], 0)
            for idx, (bi, n, w) in enumerate(pairs):
                if 0 < idx and idx + 1 < len(pairs):
                    sem(*pairs[idx + 1])
                pv(bi, n, w)
                if w == 4 * n + 2:  # last pair of the chunk
                    epi(bi, n)
                    if (bi, n) in inject:
                        inject[(bi, n)]()
    nc.compile()
    return nc


def _mask_patterns():
    # M_s[p, f] = 1.0 where j <= i for diagonal tile s of a chunk, in the
    # PERMUTED order: tile partition p holds j_rel = 8*(p%64) + 2s + p//64;
    # chunk column f holds i_rel = 8*(f%64) + f//64.
    p = np.arange(JT)[:, None]
    f = np.arange(CHUNK)[None, :]
    j_rel = 8 * (p % 64) + p // 64
    i_rel = 8 * (f % 64) + f // 64
    import ml_dtypes
    return np.stack(
        [(j_rel + 2 * s <= i_rel) for s in range(4)]).astype(ml_dtypes.bfloat16)


def _numpy_fallback(query, key, value, mask, Wq, Wk, Wv, Wo):
    q = query @ Wq.T
    v = value @ Wv.T
    qh = q.reshape(B, H, S, DK)
    vh = v.reshape(B, H, S, DK)
    scores = np.einsum("bhqd,bhkd->bhqk", qh, qh) / math.sqrt(DK)
    scores = np.where(mask == 0, np.float32(-1e9), scores)
    scores = scores - scores.max(axis=-1, keepdims=True)
    e = np.exp(scores)
    attn = e / e.sum(axis=-1, keepdims=True)
    x = np.einsum("bhqk,bhkd->bhqd", attn, vh)
    x = x.transpose(0, 2, 1, 3).reshape(B, S, H * DK)
    return (x @ Wo.T).astype(np.float32)


def _run_device(query, value, Wq, Wv, Wo, trace=False):
    from concourse.bass_utils import run_bass_kernel_spmd

    if "nc" not in _cache:
        _cache["nc"] = _build_causal()
    nc = _cache["nc"]

    import ml_dtypes
    bf16 = ml_dtypes.bfloat16

    def pk(a):  # [In, O] -> partition-major [128, In//128, O]
        return np.ascontiguousarray(
            a.reshape(-1, 128, a.shape[-1]).transpose(1, 0, 2)).astype(bf16)

    mpat = np.ascontiguousarray(
        _mask_patterns().transpose(1, 0, 2))  # [JT, 4, CHUNK]
    wqT = pk(Wq.T)
    wvT = pk(Wv.T)
    in_maps = []
    for c in range(NCORES):
        r0 = SLAB * c
        qTc = query[:, r0:r0 + SLAB, :].transpose(0, 2, 1)  # [B, D, SLAB]
        vTc = value[:, r0:r0 + SLAB, :].transpose(0, 2, 1)
        in_maps.append({
            "qT": np.ascontiguousarray(
                qTc.reshape(B, 4, 128, SLAB).transpose(0, 2, 1, 3)
            ).astype(bf16),
            "vT": np.ascontiguousarray(
                vTc.reshape(B, 4, 128, SLAB).transpose(0, 2, 1, 3)
            ).astype(bf16),
            "wqT": wqT,
            "wvT": wvT,
            "woT": np.ascontiguousarray(
                Wo[:, DK * c:DK * (c + 1)].T).astype(bf16),
            "mpat": mpat,
        })
    res = run_bass_kernel_spmd(nc, in_maps, core_ids=list(range(NCORES)),
                               trace=trace)
    out = np.zeros((B, S, D), dtype=np.float64)
    for c in range(NCORES):
        pc = res.results[c]
        out += pc["po"].astype(np.float64) / \
            pc["lo"].reshape(B, S, 1).astype(np.float64)
    # undo the chunk-local permutation: storage (n, jj, r) holds seq 512n+8r+jj
    out = out.reshape(B, NCHUNK, 8, 64, D).transpose(0, 1, 3, 2, 4) \
             .reshape(B, S, D)
    return out.astype(np.float32), res


_TRIL = None


def kernel(query, key, value, mask, Wq, Wk, Wv, Wo):
    global _TRIL
    query = np.asarray(query, dtype=np.float32)
    value = np.asarray(value, dtype=np.float32)
    mask = np.asarray(mask)
    Wq = np.asarray(Wq, dtype=np.float32)
    Wv = np.asarray(Wv, dtype=np.float32)
    Wo = np.asarray(Wo, dtype=np.float32)

    if _TRIL is None:
        _TRIL = np.tril(np.ones((S, S), dtype=np.int64))
    m2 = mask.reshape(S, S)
    if not np.array_equal(m2 != 0, _TRIL != 0):
        return _numpy_fallback(query, np.asarray(key), value, mask,
                               Wq, np.asarray(Wk), Wv, Wo)

    out, _ = _run_device(query, value, Wq, Wv, Wo)
    return out



# revision 46
# speedup vs baseline: 1.0493x; 1.0493x over previous
"""Multi-head attention (with the repo's k=q bug) on 8 Trainium2 NeuronCores.

Reference computation (B=2, S=2048, D=512, H=8, DK=64):
    q = query @ Wq.T ; v = value @ Wv.T          (k-projection is dead code)
    qh = q.reshape(B, H, S, DK)  (raw view: head h = a contiguous 256-row slab
                                  of q, re-chunked into rows of 64)
    kh = qh                      (repo bug: key = query.view(...))
    scores = qh @ qh^T / 8 ; mask ; softmax ; x = attn @ vh
    out = x.transpose/reshape @ Wo.T

Sharding: core c owns head h=c for both batches (2 (b,h) pairs/core).
Per pair the device computes, in score-TRANSPOSED layout S_T[j, i]:
    E_T = exp(S_T/8 - 20) * mask_T        (mask synthesized host-side; only
                                           diagonal blocks need one)
    [x_unnorm^T; l] = [vh | ones]^T-style augmented PV matmul
    po = x_unnorm^T.T @ Wo_h.T            (unnormalized partial output)
The host divides po rows by l and sums partials over heads/cores.
The causal structure of the mask (verified exactly on the host) lets the
device skip all fully-masked j-tiles. Non-causal masks fall back to numpy.
"""

import math
import sys

import numpy as np

sys.path.insert(0, "/opt/trn_rl_repo")

B, S, D, H, DK = 2, 2048, 512, 8, 64
NCORES = 8
SLAB = S // H          # 256 query rows per head-slab
CHUNK = 512            # i-chunk width (PSUM bank)
JT = 128               # j-tile height
NCHUNK = S // CHUNK    # 4
NJT = S // JT          # 16
EXP_BIAS = -20.0
RESTRICT_S = False
RESTRICT_PV = False       # exp(s/8 - 20): overflow headroom; cancels in l-division


def _enable_ldw_opt():
    """Flip walrus --enable-ldw-opt: elides back-to-back LDWEIGHTS of the
    same stationary operand, which the t-grouped matmul order produces."""
    from concourse import bass_utils
    if getattr(bass_utils, "_ldw_patched", False):
        return
    orig = bass_utils.run_command

    def run_command(argv, **kw):
        argv = [a.replace("--enable-ldw-opt=false", "--enable-ldw-opt=true")
                if isinstance(a, str) else a for a in argv]
        return orig(argv, **kw)

    bass_utils.run_command = run_command
    bass_utils._ldw_patched = True

_cache: dict = {}


def _build_causal():
    import concourse.bass as bass
    import concourse.tile as tile
    from concourse import bacc, mybir

    f32 = mybir.dt.float32
    bf16 = mybir.dt.bfloat16
    nc = bacc.Bacc("TRN2", target_bir_lowering=False, debug=False,
                   num_devices=NCORES)

    # inputs are host-prearranged partition-major ([128, k, o] contiguous)
    # so each input loads with one DMA of 2-4KB-per-partition descriptors
    KTc = D // 128
    qT = nc.dram_tensor("qT", [B, 128, KTc, SLAB], bf16,
                        kind="ExternalInput").ap()
    vT = nc.dram_tensor("vT", [B, 128, KTc, SLAB], bf16,
                        kind="ExternalInput").ap()
    wqT = nc.dram_tensor("wqT", [128, KTc, D], bf16,
                         kind="ExternalInput").ap()
    wvT = nc.dram_tensor("wvT", [128, KTc, D], bf16,
                         kind="ExternalInput").ap()
    woT = nc.dram_tensor("woT", [DK, D], bf16, kind="ExternalInput").ap()
    mpat = nc.dram_tensor("mpat", [JT, 4, CHUNK], mybir.dt.bfloat16,
                          kind="ExternalInput").ap()
    po = nc.dram_tensor("po", [B, S, D], bf16, kind="ExternalOutput").ap()
    lo = nc.dram_tensor("lo", [B, 1, S], f32, kind="ExternalOutput").ap()

    KT = D // 128  # 4 k-tiles over the contraction dim of the projections

    def mm(out, lhsT, rhs, **kw):
        nc.tensor.matmul(out, lhsT, rhs, **kw)

    def r(ap):
        return ap

    with tile.TileContext(nc) as tc:
        with (
            tc.tile_pool(name="const", bufs=1) as constp,
            tc.tile_pool(name="acts", bufs=2) as actp,
            tc.tile_pool(name="qhT", bufs=2) as qhTp,
            tc.tile_pool(name="vh", bufs=2) as vhp,
            tc.tile_pool(name="eT", bufs=4) as eTp,
            tc.tile_pool(name="xT", bufs=2) as xTp,
            tc.tile_pool(name="fo", bufs=3) as fop,
            tc.tile_pool(name="psS", bufs=2, space="PSUM") as psS,
            tc.tile_pool(name="psX", bufs=2, space="PSUM") as psX,
            tc.tile_pool(name="psM", bufs=2, space="PSUM") as psM,
        ):
            # --- all DMA loads upfront, in first-needed order ------------
            # sync: wq k-tiles; scalar: qT (bi=0) -- the first projection's
            # inputs land first; everything else queues behind.
            wq_sb, wv_sb = [], []
            qT_sbs = [[None] * KT for _ in range(B)]
            vT_sbs = [[None] * KT for _ in range(B)]
            # batched input loads, partition-major host layouts; the ramp-
            # critical wq+qT0 are split across the three issue queues so
            # their ~110GB/s per-queue rates add up
            wqa = constp.tile([128, KT, D], bf16, tag="wqa")
            nc.sync.dma_start(wqa[:, 0:2, :], wqT[:, 0:2, :])
            nc.scalar.dma_start(wqa[:, 2:4, :], wqT[:, 2:4, :])
            wq_sb = [wqa[:, k, :] for k in range(KT)]
            qta0 = actp.tile([128, KT, SLAB], bf16, tag="qta")
            nc.scalar.dma_start(qta0[:, 0:2, :], qT[0][:, 0:2, :])
            nc.sync.dma_start(qta0[:, 2:4, :], qT[0][:, 2:4, :])
            qT_sbs[0] = [qta0[:, k, :] for k in range(KT)]
            wva = constp.tile([128, KT, D], bf16, tag="wva")
            nc.gpsimd.dma_start(wva[:], wvT[:])
            wv_sb = [wva[:, k, :] for k in range(KT)]
            vta0 = actp.tile([128, KT, SLAB], bf16, tag="vta")
            nc.sync.dma_start(vta0[:], vT[0])
            vT_sbs[0] = [vta0[:, k, :] for k in range(KT)]
            qta1 = actp.tile([128, KT, SLAB], bf16, tag="qta")
            nc.scalar.dma_start(qta1[:], qT[1])
            qT_sbs[1] = [qta1[:, k, :] for k in range(KT)]
            vta1 = actp.tile([128, KT, SLAB], bf16, tag="vta")
            nc.sync.dma_start(vta1[:], vT[1])
            vT_sbs[1] = [vta1[:, k, :] for k in range(KT)]
            # Wo^T slice duplicated into both partition halves (row-packing)
            wo_sb = constp.tile([128, D], bf16, tag="wo")
            nc.gpsimd.dma_start(r(wo_sb[0:64, :]), r(woT[:, :]))
            nc.gpsimd.dma_start(r(wo_sb[64:128, :]), r(woT[:, :]))
            mpa = constp.tile([JT, 4, CHUNK], bf16, tag="mpa")
            nc.gpsimd.dma_start(mpa[:], mpat[:])
            mp_sb = [mpa[:, s_, :] for s_ in range(4)]
            exp_bias = constp.tile([128, 1], f32, tag="ebias")
            nc.gpsimd.memset(exp_bias[:], EXP_BIAS)

            # --- projection emitters ---------------------------------------
            # PERMUTED sequence order: storage position n*512 + jj*64 + r
            # holds seq = 512n + 8r + jj (chunk-local permutation). Makes
            # every matmul operand a contiguous slice and the PSUM->SBUF
            # casts near-contiguous (4 runs of 64). Host unpermutes rows.
            def q_proj_part(bi, qhT, jgs):
                qT_sb = qT_sbs[bi]
                qhT_v = qhT.rearrange("p (n j r) -> p n j r", n=NCHUNK, j=H)
                # dup for K=64 row-packing: scalar is idle during the b0 ramp
                # but exp-bound once attention runs (b1's projections)
                for jg in jgs:
                    ps = psM.tile([128, SLAB], f32, tag="psm")
                    for k in range(KT):
                        mm(
                            ps[:], wq_sb[k][:, 128 * jg:128 * (jg + 1)],
                            qT_sb[k][:], start=(k == 0), stop=(k == KT - 1))
                    for jl in range(2):
                        jj = 2 * jg + jl
                        nc.vector.tensor_copy(
                            qhT_v[0:64, :, jj, :], ps[64 * jl:64 * jl + 64, :])
                        if bi == 0:
                            # parallel PSUM read on the (ramp-idle) scalar
                            nc.scalar.copy(
                                qhT_v[64:128, :, jj, :],
                                ps[64 * jl:64 * jl + 64, :])
                        else:
                            nc.vector.tensor_copy(
                                qhT_v[64:128, :, jj, :], qhT_v[0:64, :, jj, :])
                return qhT

            def v_proj_part(bi, vhc, rh):
                # v slab half rh, then chunk-combined vh tiles for chunks
                # nj = 2*rh, 2*rh+1: [128, 4*(DK+1)], tile t block at cols
                # 65*(t%4); partition p = jl*64 + r_l holds
                # seq j = 512*(t//4) + 8*(p%64) + 2*(t%4) + p//64
                vT_sb = vT_sbs[bi]
                psv = psM.tile([128, D], f32, tag="psm")
                for k in range(KT):
                    mm(
                        psv[:], vT_sb[k][:, 128 * rh:128 * (rh + 1)],
                        wv_sb[k][:], start=(k == 0), stop=(k == KT - 1))
                vsl = actp.tile([128, D], bf16, tag=f"vsl{rh}")
                nc.vector.tensor_copy(vsl[:], psv[:])
                src = vsl.rearrange("p (s x) -> p s x", s=4)
                for nj in (2 * rh, 2 * rh + 1):
                    vc = vhp.tile([128, 4 * (DK + 1)], bf16, tag=f"vhc{nj}")
                    vc_r = vc.rearrange("p (t k) -> p t k", t=4)
                    nc.gpsimd.memset(vc_r[:, :, DK:DK + 1], 1.0)
                    row0 = 64 * (nj % 2)
                    nc.gpsimd.dma_start(
                        vc_r[0:64, :, 0:DK],
                        src[row0:row0 + 64, :, 0:64])
                    nc.gpsimd.dma_start(
                        vc_r[64:128, :, 0:DK],
                        src[row0:row0 + 64, :, 64:128])
                    vhc.append(vc)
                return vhc

            # --- pipelined attention over both batches --------------------
            # Pairs of j-tiles flow through: scores+exp+mask (sem) one pair
            # ahead of the PV matmuls, so the next chunk's exp is issued
            # before the current chunk's epilogue blocks the tensor queue.
            qhTs, vhcs = {}, {}
            eTs, psxs = {}, {}

            def sem(bi, n, w):
                qhT = qhTs[bi]
                pss = psS.tile([128, 2 * CHUNK], f32, tag="pss")
                for tw in range(2):
                    t_ = w + tw
                    p0 = 64 * tw
                    mm(
                        pss[:, CHUNK * tw:CHUNK * (tw + 1)],
                        qhT[p0:p0 + 64, JT * t_:JT * (t_ + 1)],
                        qhT[p0:p0 + 64, CHUNK * n:CHUNK * (n + 1)],
                        start=True, stop=True,
                        tile_position=(p0, 0))
                eT = eTp.tile([128, 2 * CHUNK], bf16, tag="eT")
                nc.scalar.activation(
                    eT[:], pss[:],
                    mybir.ActivationFunctionType.Exp,
                    bias=exp_bias[:], scale=1.0 / math.sqrt(DK))
                for tw in range(2):
                    s_ = w + tw - 4 * n
                    if s_ >= 0:
                        sl = eT[:, CHUNK * tw:CHUNK * (tw + 1)]
                        nc.vector.tensor_mul(sl, sl, mp_sb[s_][:])
                eTs[(bi, n, w)] = eT

            def pv(bi, n, w):
                n_t = 4 * n + 4
                if (bi, n) not in psxs:
                    psx = psX.tile([DK + 1, CHUNK], f32, tag="psx")
                    psxs[(bi, n)] = psx
                psx = psxs[(bi, n)]
                eT = eTs.pop((bi, n, w))
                for tw in range(2):
                    t_ = w + tw
                    vt = vhcs[bi][t_ // 4]
                    c0 = (DK + 1) * (t_ % 4)
                    nc.tensor.matmul(
                        psx[:],
                        vt[:, c0:c0 + DK + 1],
                        eT[:, CHUNK * tw:CHUNK * (tw + 1)],
                        start=(t_ == 0), stop=(t_ == n_t - 1),
                        skip_group_check=True)

            def epi(bi, n):
                psx = psxs.pop((bi, n))
                xT = xTp.tile([128, CHUNK], bf16, tag="xT")
                nc.vector.tensor_copy(xT[0:64, :], psx[0:64, :])
                nc.vector.tensor_copy(xT[64:128, :], xT[0:64, :])
                lsb = xTp.tile([1, CHUNK], f32, tag="lsb")
                nc.vector.tensor_copy(lsb[:], psx[64:65, :])
                nc.gpsimd.dma_start(
                    lo[bi, :, CHUNK * n:CHUNK * (n + 1)], lsb[:])
                for u in range(CHUNK // 128):
                    p0 = 64 * (u % 2)
                    psf = psM.tile([128, D], f32, tag="psm")
                    mm(
                        psf[:], xT[p0:p0 + 64, 128 * u:128 * (u + 1)],
                        wo_sb[p0:p0 + 64, :], start=True, stop=True,
                        tile_position=(p0, 0))
                    fo = fop.tile([128, D], bf16, tag="fo")
                    nc.vector.tensor_copy(fo[:], psf[:])
                    r0 = CHUNK * n + 128 * u
                    nc.sync.dma_start(po[bi, r0:r0 + 128, :], fo[:])

            # b1's projections are emitted in small slices between b0's later
            # (bigger) attention chunks so the exp stream can hide the bursts
            qhT_b1 = qhTp.tile([128, S], bf16, tag="qhT")
            qhTs[1] = qhT_b1
            vhcs[1] = []
            inject = {
                (0, 1): lambda: q_proj_part(1, qhTs[1], [0, 1]),
                (0, 2): lambda: (q_proj_part(1, qhTs[1], [2, 3]),
                                 v_proj_part(1, vhcs[1], 0)),
                (0, 3): lambda: v_proj_part(1, vhcs[1], 1),
            }
            qhT_b0 = qhTp.tile([128, S], bf16, tag="qhT")
            qhTs[0] = qhT_b0
            q_proj_part(0, qhTs[0], [0, 1, 2, 3])
            vhcs[0] = []
            # b1 starts with chunk 1 so the b0->b1 boundary (where b1's
            # leftover projections execute) has a meatier exp stream
            pairs = [(0, n, w) for n in (0, 1, 2, 3)
                     for w in range(0, 4 * n + 4, 2)]
            pairs += [(1, n, w) for n in (1, 2, 3, 0)
                      for w in range(0, 4 * n + 4, 2)]
            # sems run 1 pair ahead of the PV matmuls
            v_proj_part(0, vhcs[0], 0)
            v_proj_part(0, vhcs[0], 1)
            sem(*pairs[0])
            for idx, (bi, n, w) in enumerate(pairs):
                if idx + 1 < len(pairs):
                    sem(*pairs[idx + 1])
                pv(bi, n, w)
                if w == 4 * n + 2:  # last pair of the chunk
                    epi(bi, n)
                    if (bi, n) in inject:
                        inject[(bi, n)]()
    nc.compile()
    return nc


def _mask_patterns():
    # M_s[p, f] = 1.0 where j <= i for diagonal tile s of a chunk, in the
    # PERMUTED order: tile partition p holds j_rel = 8*(p%64) + 2s + p//64;
    # chunk column f holds i_rel = 8*(f%64) + f//64.
    p = np.arange(JT)[:, None]
    f = np.arange(CHUNK)[None, :]
    j_rel = 8 * (p % 64) + p // 64
    i_rel = 8 * (f % 64) + f // 64
    import ml_dtypes
    return np.stack(
        [(j_rel + 2 * s <= i_rel) for s in range(4)]).astype(ml_dtypes.bfloat16)


def _numpy_fallback(query, key, value, mask, Wq, Wk, Wv, Wo):
    q = query @ Wq.T
    v = value @ Wv.T
    qh = q.reshape(B, H, S, DK)
    vh = v.reshape(B, H, S, DK)
    scores = np.einsum("bhqd,bhkd->bhqk", qh, qh) / math.sqrt(DK)
    scores = np.where(mask == 0, np.float32(-1e9), scores)
    scores = scores - scores.max(axis=-1, keepdims=True)
    e = np.exp(scores)
    attn = e / e.sum(axis=-1, keepdims=True)
    x = np.einsum("bhqk,bhkd->bhqd", attn, vh)
    x = x.transpose(0, 2, 1, 3).reshape(B, S, H * DK)
    return (x @ Wo.T).astype(np.float32)


def _run_device(query, value, Wq, Wv, Wo, trace=False):
    from concourse.bass_utils import run_bass_kernel_spmd

    if "nc" not in _cache:
        _cache["nc"] = _build_causal()
    nc = _cache["nc"]

    import ml_dtypes
    bf16 = ml_dtypes.bfloat16

    def pk(a):  # [In, O] -> partition-major [128, In//128, O]
        return np.ascontiguousarray(
            a.reshape(-1, 128, a.shape[-1]).transpose(1, 0, 2)).astype(bf16)

    mpat = np.ascontiguousarray(
        _mask_patterns().transpose(1, 0, 2))  # [JT, 4, CHUNK]
    wqT = pk(Wq.T)
    wvT = pk(Wv.T)
    in_maps = []
    for c in range(NCORES):
        r0 = SLAB * c
        qTc = query[:, r0:r0 + SLAB, :].transpose(0, 2, 1)  # [B, D, SLAB]
        vTc = value[:, r0:r0 + SLAB, :].transpose(0, 2, 1)
        in_maps.append({
            "qT": np.ascontiguousarray(
                qTc.reshape(B, 4, 128, SLAB).transpose(0, 2, 1, 3)
            ).astype(bf16),
            "vT": np.ascontiguousarray(
                vTc.reshape(B, 4, 128, SLAB).transpose(0, 2, 1, 3)
            ).astype(bf16),
            "wqT": wqT,
            "wvT": wvT,
            "woT": np.ascontiguousarray(
                Wo[:, DK * c:DK * (c + 1)].T).astype(bf16),
            "mpat": mpat,
        })
    res = run_bass_kernel_spmd(nc, in_maps, core_ids=list(range(NCORES)),
                               trace=trace)
    out = np.zeros((B, S, D), dtype=np.float64)
    for c in range(NCORES):
        pc = res.results[c]
        out += pc["po"].astype(np.float64) / \
            pc["lo"].reshape(B, S, 1).astype(np.float64)
    # undo the chunk-local permutation: storage (n, jj, r) holds seq 512n+8r+jj
    out = out.reshape(B, NCHUNK, 8, 64, D).transpose(0, 1, 3, 2, 4) \
             .reshape(B, S, D)
    return out.astype(np.float32), res


_TRIL = None


def kernel(query, key, value, mask, Wq, Wk, Wv, Wo):
    global _TRIL
    query = np.asarray(query, dtype=np.float32)
    value = np.asarray(value, dtype=np.float32)
    mask = np.asarray(mask)
    Wq = np.asarray(Wq, dtype=np.float32)
    Wv = np.asarray(Wv, dtype=np.float32)
    Wo = np.asarray(Wo, dtype=np.float32)

    if _TRIL is None:
        _TRIL = np.tril(np.ones((S, S), dtype=np.int64))
    m2 = mask.reshape(S, S)
    if not np.array_equal(m2 != 0, _TRIL != 0):
        return _numpy_fallback(query, np.asarray(key), value, mask,
                               Wq, np.asarray(Wk), Wv, Wo)

    out, _ = _run_device(query, value, Wq, Wv, Wo)
    return out



# revision 47
# speedup vs baseline: 1.0729x; 1.0225x over previous
"""Multi-head attention (with the repo's k=q bug) on 8 Trainium2 NeuronCores.

Reference computation (B=2, S=2048, D=512, H=8, DK=64):
    q = query @ Wq.T ; v = value @ Wv.T          (k-projection is dead code)
    qh = q.reshape(B, H, S, DK)  (raw view: head h = a contiguous 256-row slab
                                  of q, re-chunked into rows of 64)
    kh = qh                      (repo bug: key = query.view(...))
    scores = qh @ qh^T / 8 ; mask ; softmax ; x = attn @ vh
    out = x.transpose/reshape @ Wo.T

Sharding: core c owns head h=c for both batches (2 (b,h) pairs/core).
Per pair the device computes, in score-TRANSPOSED layout S_T[j, i]:
    E_T = exp(S_T/8 - 20) * mask_T        (mask synthesized host-side; only
                                           diagonal blocks need one)
    [x_unnorm^T; l] = [vh | ones]^T-style augmented PV matmul
    po = x_unnorm^T.T @ Wo_h.T            (unnormalized partial output)
The host divides po rows by l and sums partials over heads/cores.
The causal structure of the mask (verified exactly on the host) lets the
device skip all fully-masked j-tiles. Non-causal masks fall back to numpy.
"""

import math
import sys

import numpy as np

sys.path.insert(0, "/opt/trn_rl_repo")

B, S, D, H, DK = 2, 2048, 512, 8, 64
NCORES = 8
SLAB = S // H          # 256 query rows per head-slab
CHUNK = 512            # i-chunk width (PSUM bank)
JT = 128               # j-tile height
NCHUNK = S // CHUNK    # 4
NJT = S // JT          # 16
EXP_BIAS = -20.0
RESTRICT_S = False
RESTRICT_PV = False       # exp(s/8 - 20): overflow headroom; cancels in l-division


def _enable_ldw_opt():
    """Flip walrus --enable-ldw-opt: elides back-to-back LDWEIGHTS of the
    same stationary operand, which the t-grouped matmul order produces."""
    from concourse import bass_utils
    if getattr(bass_utils, "_ldw_patched", False):
        return
    orig = bass_utils.run_command

    def run_command(argv, **kw):
        argv = [a.replace("--enable-ldw-opt=false", "--enable-ldw-opt=true")
                if isinstance(a, str) else a for a in argv]
        return orig(argv, **kw)

    bass_utils.run_command = run_command
    bass_utils._ldw_patched = True

_cache: dict = {}


def _build_causal():
    import concourse.bass as bass
    import concourse.tile as tile
    from concourse import bacc, mybir

    f32 = mybir.dt.float32
    bf16 = mybir.dt.bfloat16
    nc = bacc.Bacc("TRN2", target_bir_lowering=False, debug=False,
                   num_devices=NCORES)

    # inputs are host-prearranged partition-major ([128, k, o] contiguous)
    # so each input loads with one DMA of 2-4KB-per-partition descriptors
    KTc = D // 128
    qT = nc.dram_tensor("qT", [B, 128, KTc, SLAB], bf16,
                        kind="ExternalInput").ap()
    vT = nc.dram_tensor("vT", [B, 128, KTc, SLAB], bf16,
                        kind="ExternalInput").ap()
    wqT = nc.dram_tensor("wqT", [128, KTc, D], bf16,
                         kind="ExternalInput").ap()
    wvT = nc.dram_tensor("wvT", [128, KTc, D], bf16,
                         kind="ExternalInput").ap()
    woT = nc.dram_tensor("woT", [DK, D], bf16, kind="ExternalInput").ap()
    mpat = nc.dram_tensor("mpat", [JT, 4, CHUNK], mybir.dt.bfloat16,
                          kind="ExternalInput").ap()
    po = nc.dram_tensor("po", [B, S, D], bf16, kind="ExternalOutput").ap()
    lo = nc.dram_tensor("lo", [B, 1, S], f32, kind="ExternalOutput").ap()

    KT = D // 128  # 4 k-tiles over the contraction dim of the projections

    def mm(out, lhsT, rhs, **kw):
        nc.tensor.matmul(out, lhsT, rhs, **kw)

    def r(ap):
        return ap

    with tile.TileContext(nc) as tc:
        with (
            tc.tile_pool(name="const", bufs=1) as constp,
            tc.tile_pool(name="acts", bufs=2) as actp,
            tc.tile_pool(name="qhT", bufs=2) as qhTp,
            tc.tile_pool(name="vh", bufs=2) as vhp,
            tc.tile_pool(name="eT", bufs=4) as eTp,
            tc.tile_pool(name="xT", bufs=2) as xTp,
            tc.tile_pool(name="fo", bufs=3) as fop,
            tc.tile_pool(name="psS", bufs=2, space="PSUM") as psS,
            tc.tile_pool(name="psX", bufs=2, space="PSUM") as psX,
            tc.tile_pool(name="psM", bufs=2, space="PSUM") as psM,
        ):
            # --- all DMA loads upfront, in first-needed order ------------
            # sync: wq k-tiles; scalar: qT (bi=0) -- the first projection's
            # inputs land first; everything else queues behind.
            wq_sb, wv_sb = [], []
            qT_sbs = [[None] * KT for _ in range(B)]
            vT_sbs = [[None] * KT for _ in range(B)]
            # batched input loads, partition-major host layouts. Three issue
            # queues (~110GB/s each) in waves ordered by first use:
            #   wave 1: wq halves (sync+gpsimd) and qT0 (scalar)
            #   wave 2: vT0, mask patterns, wv
            #   wave 3: batch-1 inputs and Wo
            wqa = constp.tile([128, KT, D], bf16, tag="wqa")
            nc.sync.dma_start(wqa[:, 0:2, :], wqT[:, 0:2, :])
            nc.gpsimd.dma_start(wqa[:, 2:4, :], wqT[:, 2:4, :])
            wq_sb = [wqa[:, k, :] for k in range(KT)]
            qta0 = actp.tile([128, KT, SLAB], bf16, tag="qta")
            nc.scalar.dma_start(qta0[:], qT[0])
            qT_sbs[0] = [qta0[:, k, :] for k in range(KT)]
            vta0 = actp.tile([128, KT, SLAB], bf16, tag="vta")
            nc.sync.dma_start(vta0[:], vT[0])
            vT_sbs[0] = [vta0[:, k, :] for k in range(KT)]
            mpa = constp.tile([JT, 4, CHUNK], bf16, tag="mpa")
            nc.scalar.dma_start(mpa[:], mpat[:])
            mp_sb = [mpa[:, s_, :] for s_ in range(4)]
            wva = constp.tile([128, KT, D], bf16, tag="wva")
            nc.gpsimd.dma_start(wva[:], wvT[:])
            wv_sb = [wva[:, k, :] for k in range(KT)]
            qta1 = actp.tile([128, KT, SLAB], bf16, tag="qta")
            nc.sync.dma_start(qta1[:], qT[1])
            qT_sbs[1] = [qta1[:, k, :] for k in range(KT)]
            vta1 = actp.tile([128, KT, SLAB], bf16, tag="vta")
            nc.scalar.dma_start(vta1[:], vT[1])
            vT_sbs[1] = [vta1[:, k, :] for k in range(KT)]
            # Wo^T slice duplicated into both partition halves (row-packing)
            wo_sb = constp.tile([128, D], bf16, tag="wo")
            nc.gpsimd.dma_start(r(wo_sb[0:64, :]), r(woT[:, :]))
            nc.gpsimd.dma_start(r(wo_sb[64:128, :]), r(woT[:, :]))
            exp_bias = constp.tile([128, 1], f32, tag="ebias")
            nc.gpsimd.memset(exp_bias[:], EXP_BIAS)

            # --- projection emitters ---------------------------------------
            # PERMUTED sequence order: storage position n*512 + jj*64 + r
            # holds seq = 512n + 8r + jj (chunk-local permutation). Makes
            # every matmul operand a contiguous slice and the PSUM->SBUF
            # casts near-contiguous (4 runs of 64). Host unpermutes rows.
            def q_proj_part(bi, qhT, jgs):
                qT_sb = qT_sbs[bi]
                qhT_v = qhT.rearrange("p (n j r) -> p n j r", n=NCHUNK, j=H)
                # dup for K=64 row-packing: scalar is idle during the b0 ramp
                # but exp-bound once attention runs (b1's projections)
                for jg in jgs:
                    ps = psM.tile([128, SLAB], f32, tag="psm")
                    for k in range(KT):
                        mm(
                            ps[:], wq_sb[k][:, 128 * jg:128 * (jg + 1)],
                            qT_sb[k][:], start=(k == 0), stop=(k == KT - 1))
                    for jl in range(2):
                        jj = 2 * jg + jl
                        nc.vector.tensor_copy(
                            qhT_v[0:64, :, jj, :], ps[64 * jl:64 * jl + 64, :])
                        if bi == 0:
                            # parallel PSUM read on the (ramp-idle) scalar
                            nc.scalar.copy(
                                qhT_v[64:128, :, jj, :],
                                ps[64 * jl:64 * jl + 64, :])
                        else:
                            nc.vector.tensor_copy(
                                qhT_v[64:128, :, jj, :], qhT_v[0:64, :, jj, :])
                return qhT

            def v_proj_part(bi, vhc, rh):
                # v slab half rh, then chunk-combined vh tiles for chunks
                # nj = 2*rh, 2*rh+1: [128, 4*(DK+1)], tile t block at cols
                # 65*(t%4); partition p = jl*64 + r_l holds
                # seq j = 512*(t//4) + 8*(p%64) + 2*(t%4) + p//64
                vT_sb = vT_sbs[bi]
                psv = psM.tile([128, D], f32, tag="psm")
                for k in range(KT):
                    mm(
                        psv[:], vT_sb[k][:, 128 * rh:128 * (rh + 1)],
                        wv_sb[k][:], start=(k == 0), stop=(k == KT - 1))
                vsl = actp.tile([128, D], bf16, tag=f"vsl{rh}")
                nc.vector.tensor_copy(vsl[:], psv[:])
                src = vsl.rearrange("p (s x) -> p s x", s=4)
                for nj in (2 * rh, 2 * rh + 1):
                    vc = vhp.tile([128, 4 * (DK + 1)], bf16, tag=f"vhc{nj}")
                    vc_r = vc.rearrange("p (t k) -> p t k", t=4)
                    nc.gpsimd.memset(vc_r[:, :, DK:DK + 1], 1.0)
                    row0 = 64 * (nj % 2)
                    nc.gpsimd.dma_start(
                        vc_r[0:64, :, 0:DK],
                        src[row0:row0 + 64, :, 0:64])
                    nc.gpsimd.dma_start(
                        vc_r[64:128, :, 0:DK],
                        src[row0:row0 + 64, :, 64:128])
                    vhc.append(vc)
                return vhc

            # --- pipelined attention over both batches --------------------
            # Pairs of j-tiles flow through: scores+exp+mask (sem) one pair
            # ahead of the PV matmuls, so the next chunk's exp is issued
            # before the current chunk's epilogue blocks the tensor queue.
            qhTs, vhcs = {}, {}
            eTs, psxs = {}, {}

            def sem(bi, n, w):
                qhT = qhTs[bi]
                pss = psS.tile([128, 2 * CHUNK], f32, tag="pss")
                for tw in range(2):
                    t_ = w + tw
                    p0 = 64 * tw
                    mm(
                        pss[:, CHUNK * tw:CHUNK * (tw + 1)],
                        qhT[p0:p0 + 64, JT * t_:JT * (t_ + 1)],
                        qhT[p0:p0 + 64, CHUNK * n:CHUNK * (n + 1)],
                        start=True, stop=True,
                        tile_position=(p0, 0))
                eT = eTp.tile([128, 2 * CHUNK], bf16, tag="eT")
                nc.scalar.activation(
                    eT[:], pss[:],
                    mybir.ActivationFunctionType.Exp,
                    bias=exp_bias[:], scale=1.0 / math.sqrt(DK))
                for tw in range(2):
                    s_ = w + tw - 4 * n
                    if s_ >= 0:
                        sl = eT[:, CHUNK * tw:CHUNK * (tw + 1)]
                        nc.vector.tensor_mul(sl, sl, mp_sb[s_][:])
                eTs[(bi, n, w)] = eT

            def pv(bi, n, w):
                n_t = 4 * n + 4
                if (bi, n) not in psxs:
                    psx = psX.tile([DK + 1, CHUNK], f32, tag="psx")
                    psxs[(bi, n)] = psx
                psx = psxs[(bi, n)]
                eT = eTs.pop((bi, n, w))
                for tw in range(2):
                    t_ = w + tw
                    vt = vhcs[bi][t_ // 4]
                    c0 = (DK + 1) * (t_ % 4)
                    nc.tensor.matmul(
                        psx[:],
                        vt[:, c0:c0 + DK + 1],
                        eT[:, CHUNK * tw:CHUNK * (tw + 1)],
                        start=(t_ == 0), stop=(t_ == n_t - 1),
                        skip_group_check=True)

            def epi(bi, n):
                psx = psxs.pop((bi, n))
                xT = xTp.tile([128, CHUNK], bf16, tag="xT")
                nc.vector.tensor_copy(xT[0:64, :], psx[0:64, :])
                nc.vector.tensor_copy(xT[64:128, :], xT[0:64, :])
                lsb = xTp.tile([1, CHUNK], f32, tag="lsb")
                nc.vector.tensor_copy(lsb[:], psx[64:65, :])
                nc.gpsimd.dma_start(
                    lo[bi, :, CHUNK * n:CHUNK * (n + 1)], lsb[:])
                for u in range(CHUNK // 128):
                    p0 = 64 * (u % 2)
                    psf = psM.tile([128, D], f32, tag="psm")
                    mm(
                        psf[:], xT[p0:p0 + 64, 128 * u:128 * (u + 1)],
                        wo_sb[p0:p0 + 64, :], start=True, stop=True,
                        tile_position=(p0, 0))
                    fo = fop.tile([128, D], bf16, tag="fo")
                    nc.vector.tensor_copy(fo[:], psf[:])
                    r0 = CHUNK * n + 128 * u
                    nc.sync.dma_start(po[bi, r0:r0 + 128, :], fo[:])

            # b1's projections are emitted in small slices between b0's later
            # (bigger) attention chunks so the exp stream can hide the bursts
            qhT_b1 = qhTp.tile([128, S], bf16, tag="qhT")
            qhTs[1] = qhT_b1
            vhcs[1] = []
            inject = {
                (0, 1): lambda: q_proj_part(1, qhTs[1], [0, 1]),
                (0, 2): lambda: (q_proj_part(1, qhTs[1], [2, 3]),
                                 v_proj_part(1, vhcs[1], 0)),
                (0, 3): lambda: v_proj_part(1, vhcs[1], 1),
            }
            qhT_b0 = qhTp.tile([128, S], bf16, tag="qhT")
            qhTs[0] = qhT_b0
            q_proj_part(0, qhTs[0], [0, 1, 2, 3])
            vhcs[0] = []
            # b1 starts with chunk 1 so the b0->b1 boundary (where b1's
            # leftover projections execute) has a meatier exp stream
            pairs = [(0, n, w) for n in (0, 1, 2, 3)
                     for w in range(0, 4 * n + 4, 2)]
            pairs += [(1, n, w) for n in (1, 2, 3, 0)
                      for w in range(0, 4 * n + 4, 2)]
            # sems run 1 pair ahead of the PV matmuls
            v_proj_part(0, vhcs[0], 0)
            v_proj_part(0, vhcs[0], 1)
            sem(*pairs[0])
            for idx, (bi, n, w) in enumerate(pairs):
                if idx + 1 < len(pairs):
                    sem(*pairs[idx + 1])
                pv(bi, n, w)
                if w == 4 * n + 2:  # last pair of the chunk
                    epi(bi, n)
                    if (bi, n) in inject:
                        inject[(bi, n)]()
    nc.compile()
    return nc


def _mask_patterns():
    # M_s[p, f] = 1.0 where j <= i for diagonal tile s of a chunk, in the
    # PERMUTED order: tile partition p holds j_rel = 8*(p%64) + 2s + p//64;
    # chunk column f holds i_rel = 8*(f%64) + f//64.
    p = np.arange(JT)[:, None]
    f = np.arange(CHUNK)[None, :]
    j_rel = 8 * (p % 64) + p // 64
    i_rel = 8 * (f % 64) + f // 64
    import ml_dtypes
    return np.stack(
        [(j_rel + 2 * s <= i_rel) for s in range(4)]).astype(ml_dtypes.bfloat16)


def _numpy_fallback(query, key, value, mask, Wq, Wk, Wv, Wo):
    q = query @ Wq.T
    v = value @ Wv.T
    qh = q.reshape(B, H, S, DK)
    vh = v.reshape(B, H, S, DK)
    scores = np.einsum("bhqd,bhkd->bhqk", qh, qh) / math.sqrt(DK)
    scores = np.where(mask == 0, np.float32(-1e9), scores)
    scores = scores - scores.max(axis=-1, keepdims=True)
    e = np.exp(scores)
    attn = e / e.sum(axis=-1, keepdims=True)
    x = np.einsum("bhqk,bhkd->bhqd", attn, vh)
    x = x.transpose(0, 2, 1, 3).reshape(B, S, H * DK)
    return (x @ Wo.T).astype(np.float32)


def _run_device(query, value, Wq, Wv, Wo, trace=False):
    from concourse.bass_utils import run_bass_kernel_spmd

    if "nc" not in _cache:
        _cache["nc"] = _build_causal()
    nc = _cache["nc"]

    import ml_dtypes
    bf16 = ml_dtypes.bfloat16

    def pk(a):  # [In, O] -> partition-major [128, In//128, O]
        return np.ascontiguousarray(
            a.reshape(-1, 128, a.shape[-1]).transpose(1, 0, 2)).astype(bf16)

    mpat = np.ascontiguousarray(
        _mask_patterns().transpose(1, 0, 2))  # [JT, 4, CHUNK]
    wqT = pk(Wq.T)
    wvT = pk(Wv.T)
    in_maps = []
    for c in range(NCORES):
        r0 = SLAB * c
        qTc = query[:, r0:r0 + SLAB, :].transpose(0, 2, 1)  # [B, D, SLAB]
        vTc = value[:, r0:r0 + SLAB, :].transpose(0, 2, 1)
        in_maps.append({
            "qT": np.ascontiguousarray(
                qTc.reshape(B, 4, 128, SLAB).transpose(0, 2, 1, 3)
            ).astype(bf16),
            "vT": np.ascontiguousarray(
                vTc.reshape(B, 4, 128, SLAB).transpose(0, 2, 1, 3)
            ).astype(bf16),
            "wqT": wqT,
            "wvT": wvT,
            "woT": np.ascontiguousarray(
                Wo[:, DK * c:DK * (c + 1)].T).astype(bf16),
            "mpat": mpat,
        })
    res = run_bass_kernel_spmd(nc, in_maps, core_ids=list(range(NCORES)),
                               trace=trace)
    out = np.zeros((B, S, D), dtype=np.float64)
    for c in range(NCORES):
        pc = res.results[c]
        out += pc["po"].astype(np.float64) / \
            pc["lo"].reshape(B, S, 1).astype(np.float64)
    # undo the chunk-local permutation: storage (n, jj, r) holds seq 512n+8r+jj
    out = out.reshape(B, NCHUNK, 8, 64, D).transpose(0, 1, 3, 2, 4) \
             .reshape(B, S, D)
    return out.astype(np.float32), res


_TRIL = None


def kernel(query, key, value, mask, Wq, Wk, Wv, Wo):
    global _TRIL
    query = np.asarray(query, dtype=np.float32)
    value = np.asarray(value, dtype=np.float32)
    mask = np.asarray(mask)
    Wq = np.asarray(Wq, dtype=np.float32)
    Wv = np.asarray(Wv, dtype=np.float32)
    Wo = np.asarray(Wo, dtype=np.float32)

    if _TRIL is None:
        _TRIL = np.tril(np.ones((S, S), dtype=np.int64))
    m2 = mask.reshape(S, S)
    if not np.array_equal(m2 != 0, _TRIL != 0):
        return _numpy_fallback(query, np.asarray(key), value, mask,
                               Wq, np.asarray(Wk), Wv, Wo)

    out, _ = _run_device(query, value, Wq, Wv, Wo)
    return out



# revision 49
# speedup vs baseline: 1.0790x; 1.0057x over previous
"""Multi-head attention (with the repo's k=q bug) on 8 Trainium2 NeuronCores.

Reference computation (B=2, S=2048, D=512, H=8, DK=64):
    q = query @ Wq.T ; v = value @ Wv.T          (k-projection is dead code)
    qh = q.reshape(B, H, S, DK)  (raw view: head h = a contiguous 256-row slab
                                  of q, re-chunked into rows of 64)
    kh = qh                      (repo bug: key = query.view(...))
    scores = qh @ qh^T / 8 ; mask ; softmax ; x = attn @ vh
    out = x.transpose/reshape @ Wo.T

Sharding: core c owns head h=c for both batches (2 (b,h) pairs/core).
Per pair the device computes, in score-TRANSPOSED layout S_T[j, i]:
    E_T = exp(S_T/8 - 20) * mask_T        (mask synthesized host-side; only
                                           diagonal blocks need one)
    [x_unnorm^T; l] = [vh | ones]^T-style augmented PV matmul
    po = x_unnorm^T.T @ Wo_h.T            (unnormalized partial output)
The host divides po rows by l and sums partials over heads/cores.
The causal structure of the mask (verified exactly on the host) lets the
device skip all fully-masked j-tiles. Non-causal masks fall back to numpy.
"""

import math
import sys

import numpy as np

sys.path.insert(0, "/opt/trn_rl_repo")

B, S, D, H, DK = 2, 2048, 512, 8, 64
NCORES = 8
SLAB = S // H          # 256 query rows per head-slab
CHUNK = 512            # i-chunk width (PSUM bank)
JT = 128               # j-tile height
NCHUNK = S // CHUNK    # 4
NJT = S // JT          # 16
EXP_BIAS = -20.0
RESTRICT_S = False
RESTRICT_PV = False       # exp(s/8 - 20): overflow headroom; cancels in l-division


def _enable_ldw_opt():
    """Flip walrus --enable-ldw-opt: elides back-to-back LDWEIGHTS of the
    same stationary operand, which the t-grouped matmul order produces."""
    from concourse import bass_utils
    if getattr(bass_utils, "_ldw_patched", False):
        return
    orig = bass_utils.run_command

    def run_command(argv, **kw):
        argv = [a.replace("--enable-ldw-opt=false", "--enable-ldw-opt=true")
                if isinstance(a, str) else a for a in argv]
        return orig(argv, **kw)

    bass_utils.run_command = run_command
    bass_utils._ldw_patched = True

_cache: dict = {}


def _build_causal():
    import concourse.bass as bass
    import concourse.tile as tile
    from concourse import bacc, mybir

    f32 = mybir.dt.float32
    bf16 = mybir.dt.bfloat16
    nc = bacc.Bacc("TRN2", target_bir_lowering=False, debug=False,
                   num_devices=NCORES)

    # inputs are host-prearranged partition-major ([128, k, o] contiguous)
    # so each input loads with one DMA of 2-4KB-per-partition descriptors
    KTc = D // 128
    qT = nc.dram_tensor("qT", [B, 128, KTc, SLAB], bf16,
                        kind="ExternalInput").ap()
    vT = nc.dram_tensor("vT", [B, 128, KTc, SLAB], bf16,
                        kind="ExternalInput").ap()
    wqT = nc.dram_tensor("wqT", [128, KTc, D], bf16,
                         kind="ExternalInput").ap()
    wvT = nc.dram_tensor("wvT", [128, KTc, D], bf16,
                         kind="ExternalInput").ap()
    woT = nc.dram_tensor("woT", [DK, D], bf16, kind="ExternalInput").ap()
    mpat = nc.dram_tensor("mpat", [JT, 4, CHUNK], mybir.dt.bfloat16,
                          kind="ExternalInput").ap()
    po = nc.dram_tensor("po", [B, S, D], bf16, kind="ExternalOutput").ap()
    lo = nc.dram_tensor("lo", [B, 1, S], f32, kind="ExternalOutput").ap()

    KT = D // 128  # 4 k-tiles over the contraction dim of the projections

    def mm(out, lhsT, rhs, **kw):
        nc.tensor.matmul(out, lhsT, rhs, **kw)

    def r(ap):
        return ap

    with tile.TileContext(nc) as tc:
        with (
            tc.tile_pool(name="const", bufs=1) as constp,
            tc.tile_pool(name="acts", bufs=2) as actp,
            tc.tile_pool(name="qhT", bufs=2) as qhTp,
            tc.tile_pool(name="vh", bufs=2) as vhp,
            tc.tile_pool(name="eT", bufs=4) as eTp,
            tc.tile_pool(name="xT", bufs=2) as xTp,
            tc.tile_pool(name="fo", bufs=3) as fop,
            tc.tile_pool(name="psS", bufs=2, space="PSUM") as psS,
            tc.tile_pool(name="psX", bufs=2, space="PSUM") as psX,
            tc.tile_pool(name="psM", bufs=2, space="PSUM") as psM,
        ):
            # --- all DMA loads upfront, in first-needed order ------------
            # sync: wq k-tiles; scalar: qT (bi=0) -- the first projection's
            # inputs land first; everything else queues behind.
            wq_sb, wv_sb = [], []
            qT_sbs = [[None] * KT for _ in range(B)]
            vT_sbs = [[None] * KT for _ in range(B)]
            # batched input loads, partition-major host layouts. Three issue
            # queues (~110GB/s each) in waves ordered by first use:
            #   wave 1: wq halves (sync+gpsimd) and qT0 (scalar)
            #   wave 2: vT0, mask patterns, wv
            #   wave 3: batch-1 inputs and Wo
            wqa = constp.tile([128, KT, D], bf16, tag="wqa")
            nc.sync.dma_start(wqa[:, 0:2, :], wqT[:, 0:2, :])
            nc.gpsimd.dma_start(wqa[:, 2:4, :], wqT[:, 2:4, :])
            wq_sb = [wqa[:, k, :] for k in range(KT)]
            qta0 = actp.tile([128, KT, SLAB], bf16, tag="qta")
            nc.scalar.dma_start(qta0[:], qT[0])
            qT_sbs[0] = [qta0[:, k, :] for k in range(KT)]
            vta0 = actp.tile([128, KT, SLAB], bf16, tag="vta")
            nc.sync.dma_start(vta0[:], vT[0])
            vT_sbs[0] = [vta0[:, k, :] for k in range(KT)]
            mpa = constp.tile([JT, 4, CHUNK], bf16, tag="mpa")
            nc.scalar.dma_start(mpa[:], mpat[:])
            mp_sb = [mpa[:, s_, :] for s_ in range(4)]
            wva = constp.tile([128, KT, D], bf16, tag="wva")
            nc.gpsimd.dma_start(wva[:], wvT[:])
            wv_sb = [wva[:, k, :] for k in range(KT)]
            qta1 = actp.tile([128, KT, SLAB], bf16, tag="qta")
            nc.sync.dma_start(qta1[:], qT[1])
            qT_sbs[1] = [qta1[:, k, :] for k in range(KT)]
            vta1 = actp.tile([128, KT, SLAB], bf16, tag="vta")
            nc.scalar.dma_start(vta1[:], vT[1])
            vT_sbs[1] = [vta1[:, k, :] for k in range(KT)]
            # Wo^T slice duplicated into both partition halves (row-packing)
            wo_sb = constp.tile([128, D], bf16, tag="wo")
            nc.gpsimd.dma_start(r(wo_sb[0:64, :]), r(woT[:, :]))
            nc.gpsimd.dma_start(r(wo_sb[64:128, :]), r(woT[:, :]))
            exp_bias = constp.tile([128, 1], f32, tag="ebias")
            nc.gpsimd.memset(exp_bias[:], EXP_BIAS)

            # --- projection emitters ---------------------------------------
            # PERMUTED sequence order: storage position n*512 + jj*64 + r
            # holds seq = 512n + 8r + jj (chunk-local permutation). Makes
            # every matmul operand a contiguous slice and the PSUM->SBUF
            # casts near-contiguous (4 runs of 64). Host unpermutes rows.
            def q_proj_part(bi, qhT, jgs):
                qT_sb = qT_sbs[bi]
                qhT_v = qhT.rearrange("p (n j r) -> p n j r", n=NCHUNK, j=H)
                # dup for K=64 row-packing: scalar is idle during the b0 ramp
                # but exp-bound once attention runs (b1's projections)
                for jg in jgs:
                    # alternate accumulator pools so evacuation of group
                    # jg-2 never stalls group jg's matmuls
                    if jg % 2 == 0:
                        ps = psM.tile([128, SLAB], f32, tag="psm")
                    else:
                        psw = psS.tile([128, 2 * CHUNK], f32, tag="pss")
                        ps = psw[:, 0:SLAB]
                    for k in range(KT):
                        mm(
                            ps[:], wq_sb[k][:, 128 * jg:128 * (jg + 1)],
                            qT_sb[k][:], start=(k == 0), stop=(k == KT - 1))
                    for jl in range(2):
                        jj = 2 * jg + jl
                        nc.vector.tensor_copy(
                            qhT_v[0:64, :, jj, :], ps[64 * jl:64 * jl + 64, :])
                        if bi == 0:
                            # parallel PSUM read on the (ramp-idle) scalar
                            nc.scalar.copy(
                                qhT_v[64:128, :, jj, :],
                                ps[64 * jl:64 * jl + 64, :])
                        else:
                            nc.vector.tensor_copy(
                                qhT_v[64:128, :, jj, :], qhT_v[0:64, :, jj, :])
                return qhT

            def v_proj_part(bi, vhc, rh):
                # v slab half rh, then chunk-combined vh tiles for chunks
                # nj = 2*rh, 2*rh+1: [128, 4*(DK+1)], tile t block at cols
                # 65*(t%4); partition p = jl*64 + r_l holds
                # seq j = 512*(t//4) + 8*(p%64) + 2*(t%4) + p//64
                vT_sb = vT_sbs[bi]
                psv = psM.tile([128, D], f32, tag="psm")
                for k in range(KT):
                    mm(
                        psv[:], vT_sb[k][:, 128 * rh:128 * (rh + 1)],
                        wv_sb[k][:], start=(k == 0), stop=(k == KT - 1))
                vsl = actp.tile([128, D], bf16, tag=f"vsl{rh}")
                nc.vector.tensor_copy(vsl[:], psv[:])
                src = vsl.rearrange("p (s x) -> p s x", s=4)
                for nj in (2 * rh, 2 * rh + 1):
                    vc = vhp.tile([128, 4 * (DK + 1)], bf16, tag=f"vhc{nj}")
                    vc_r = vc.rearrange("p (t k) -> p t k", t=4)
                    nc.gpsimd.memset(vc_r[:, :, DK:DK + 1], 1.0)
                    row0 = 64 * (nj % 2)
                    nc.gpsimd.dma_start(
                        vc_r[0:64, :, 0:DK],
                        src[row0:row0 + 64, :, 0:64])
                    nc.gpsimd.dma_start(
                        vc_r[64:128, :, 0:DK],
                        src[row0:row0 + 64, :, 64:128])
                    vhc.append(vc)
                return vhc

            # --- pipelined attention over both batches --------------------
            # Pairs of j-tiles flow through: scores+exp+mask (sem) one pair
            # ahead of the PV matmuls, so the next chunk's exp is issued
            # before the current chunk's epilogue blocks the tensor queue.
            qhTs, vhcs = {}, {}
            eTs, psxs = {}, {}

            def sem(bi, n, w):
                qhT = qhTs[bi]
                pss = psS.tile([128, 2 * CHUNK], f32, tag="pss")
                for tw in range(2):
                    t_ = w + tw
                    p0 = 64 * tw
                    mm(
                        pss[:, CHUNK * tw:CHUNK * (tw + 1)],
                        qhT[p0:p0 + 64, JT * t_:JT * (t_ + 1)],
                        qhT[p0:p0 + 64, CHUNK * n:CHUNK * (n + 1)],
                        start=True, stop=True,
                        tile_position=(p0, 0))
                eT = eTp.tile([128, 2 * CHUNK], bf16, tag="eT")
                nc.scalar.activation(
                    eT[:], pss[:],
                    mybir.ActivationFunctionType.Exp,
                    bias=exp_bias[:], scale=1.0 / math.sqrt(DK))
                for tw in range(2):
                    s_ = w + tw - 4 * n
                    if s_ >= 0:
                        sl = eT[:, CHUNK * tw:CHUNK * (tw + 1)]
                        nc.vector.tensor_mul(sl, sl, mp_sb[s_][:])
                eTs[(bi, n, w)] = eT

            def pv(bi, n, w):
                n_t = 4 * n + 4
                if (bi, n) not in psxs:
                    psx = psX.tile([DK + 1, CHUNK], f32, tag="psx")
                    psxs[(bi, n)] = psx
                psx = psxs[(bi, n)]
                eT = eTs.pop((bi, n, w))
                for tw in range(2):
                    t_ = w + tw
                    vt = vhcs[bi][t_ // 4]
                    c0 = (DK + 1) * (t_ % 4)
                    nc.tensor.matmul(
                        psx[:],
                        vt[:, c0:c0 + DK + 1],
                        eT[:, CHUNK * tw:CHUNK * (tw + 1)],
                        start=(t_ == 0), stop=(t_ == n_t - 1),
                        skip_group_check=True)

            def epi(bi, n):
                psx = psxs.pop((bi, n))
                xT = xTp.tile([128, CHUNK], bf16, tag="xT")
                nc.vector.tensor_copy(xT[0:64, :], psx[0:64, :])
                nc.vector.tensor_copy(xT[64:128, :], xT[0:64, :])
                lsb = xTp.tile([1, CHUNK], f32, tag="lsb")
                nc.vector.tensor_copy(lsb[:], psx[64:65, :])
                nc.gpsimd.dma_start(
                    lo[bi, :, CHUNK * n:CHUNK * (n + 1)], lsb[:])
                for u in range(CHUNK // 128):
                    p0 = 64 * (u % 2)
                    psf = psM.tile([128, D], f32, tag="psm")
                    mm(
                        psf[:], xT[p0:p0 + 64, 128 * u:128 * (u + 1)],
                        wo_sb[p0:p0 + 64, :], start=True, stop=True,
                        tile_position=(p0, 0))
                    fo = fop.tile([128, D], bf16, tag="fo")
                    nc.vector.tensor_copy(fo[:], psf[:])
                    r0 = CHUNK * n + 128 * u
                    nc.sync.dma_start(po[bi, r0:r0 + 128, :], fo[:])

            # b1's projections are emitted in small slices between b0's later
            # (bigger) attention chunks so the exp stream can hide the bursts
            qhT_b1 = qhTp.tile([128, S], bf16, tag="qhT")
            qhTs[1] = qhT_b1
            vhcs[1] = []
            inject = {
                (0, 1): lambda: q_proj_part(1, qhTs[1], [0, 1]),
                (0, 2): lambda: (q_proj_part(1, qhTs[1], [2, 3]),
                                 v_proj_part(1, vhcs[1], 0)),
                (0, 3): lambda: v_proj_part(1, vhcs[1], 1),
            }
            qhT_b0 = qhTp.tile([128, S], bf16, tag="qhT")
            qhTs[0] = qhT_b0
            q_proj_part(0, qhTs[0], [0, 1, 2, 3])
            vhcs[0] = []
            # b1 starts with chunk 1 so the b0->b1 boundary (where b1's
            # leftover projections execute) has a meatier exp stream
            pairs = [(0, n, w) for n in (0, 1, 2, 3)
                     for w in range(0, 4 * n + 4, 2)]
            pairs += [(1, n, w) for n in (1, 2, 3, 0)
                      for w in range(0, 4 * n + 4, 2)]
            # first two sems ahead of the v-projection so the scheduler runs
            # scores the moment qhT is ready; sems then 1 pair ahead of PV
            sem(*pairs[0])
            sem(*pairs[1])
            v_proj_part(0, vhcs[0], 0)
            v_proj_part(0, vhcs[0], 1)
            for idx, (bi, n, w) in enumerate(pairs):
                if 0 < idx and idx + 1 < len(pairs):
                    sem(*pairs[idx + 1])
                pv(bi, n, w)
                if w == 4 * n + 2:  # last pair of the chunk
                    epi(bi, n)
                    if (bi, n) in inject:
                        inject[(bi, n)]()
    nc.compile()
    return nc


def _mask_patterns():
    # M_s[p, f] = 1.0 where j <= i for diagonal tile s of a chunk, in the
    # PERMUTED order: tile partition p holds j_rel = 8*(p%64) + 2s + p//64;
    # chunk column f holds i_rel = 8*(f%64) + f//64.
    p = np.arange(JT)[:, None]
    f = np.arange(CHUNK)[None, :]
    j_rel = 8 * (p % 64) + p // 64
    i_rel = 8 * (f % 64) + f // 64
    import ml_dtypes
    return np.stack(
        [(j_rel + 2 * s <= i_rel) for s in range(4)]).astype(ml_dtypes.bfloat16)


def _numpy_fallback(query, key, value, mask, Wq, Wk, Wv, Wo):
    q = query @ Wq.T
    v = value @ Wv.T
    qh = q.reshape(B, H, S, DK)
    vh = v.reshape(B, H, S, DK)
    scores = np.einsum("bhqd,bhkd->bhqk", qh, qh) / math.sqrt(DK)
    scores = np.where(mask == 0, np.float32(-1e9), scores)
    scores = scores - scores.max(axis=-1, keepdims=True)
    e = np.exp(scores)
    attn = e / e.sum(axis=-1, keepdims=True)
    x = np.einsum("bhqk,bhkd->bhqd", attn, vh)
    x = x.transpose(0, 2, 1, 3).reshape(B, S, H * DK)
    return (x @ Wo.T).astype(np.float32)


def _run_device(query, value, Wq, Wv, Wo, trace=False):
    from concourse.bass_utils import run_bass_kernel_spmd

    if "nc" not in _cache:
        _cache["nc"] = _build_causal()
    nc = _cache["nc"]

    import ml_dtypes
    bf16 = ml_dtypes.bfloat16

    def pk(a):  # [In, O] -> partition-major [128, In//128, O]
        return np.ascontiguousarray(
            a.reshape(-1, 128, a.shape[-1]).transpose(1, 0, 2)).astype(bf16)

    mpat = np.ascontiguousarray(
        _mask_patterns().transpose(1, 0, 2))  # [JT, 4, CHUNK]
    wqT = pk(Wq.T)
    wvT = pk(Wv.T)
    in_maps = []
    for c in range(NCORES):
        r0 = SLAB * c
        qTc = query[:, r0:r0 + SLAB, :].transpose(0, 2, 1)  # [B, D, SLAB]
        vTc = value[:, r0:r0 + SLAB, :].transpose(0, 2, 1)
        in_maps.append({
            "qT": np.ascontiguousarray(
                qTc.reshape(B, 4, 128, SLAB).transpose(0, 2, 1, 3)
            ).astype(bf16),
            "vT": np.ascontiguousarray(
                vTc.reshape(B, 4, 128, SLAB).transpose(0, 2, 1, 3)
            ).astype(bf16),
            "wqT": wqT,
            "wvT": wvT,
            "woT": np.ascontiguousarray(
                Wo[:, DK * c:DK * (c + 1)].T).astype(bf16),
            "mpat": mpat,
        })
    res = run_bass_kernel_spmd(nc, in_maps, core_ids=list(range(NCORES)),
                               trace=trace)
    out = np.zeros((B, S, D), dtype=np.float64)
    for c in range(NCORES):
        pc = res.results[c]
        out += pc["po"].astype(np.float64) / \
            pc["lo"].reshape(B, S, 1).astype(np.float64)
    # undo the chunk-local permutation: storage (n, jj, r) holds seq 512n+8r+jj
    out = out.reshape(B, NCHUNK, 8, 64, D).transpose(0, 1, 3, 2, 4) \
             .reshape(B, S, D)
    return out.astype(np.float32), res


_TRIL = None


def kernel(query, key, value, mask, Wq, Wk, Wv, Wo):
    global _TRIL
    query = np.asarray(query, dtype=np.float32)
    value = np.asarray(value, dtype=np.float32)
    mask = np.asarray(mask)
    Wq = np.asarray(Wq, dtype=np.float32)
    Wv = np.asarray(Wv, dtype=np.float32)
    Wo = np.asarray(Wo, dtype=np.float32)

    if _TRIL is None:
        _TRIL = np.tril(np.ones((S, S), dtype=np.int64))
    m2 = mask.reshape(S, S)
    if not np.array_equal(m2 != 0, _TRIL != 0):
        return _numpy_fallback(query, np.asarray(key), value, mask,
                               Wq, np.asarray(Wk), Wv, Wo)

    out, _ = _run_device(query, value, Wq, Wv, Wo)
    return out

